# revision 15
# baseline (speedup 1.0000x reference)
"""DendriticBranchLayerSparse kernel for TRN2 (8 NeuronCores, batch-sharded).

out[b, o] = sum_{k<4} x[b, 4o+k] * w[4o+k]  +  t[b] * tw[o]

v7 layout: host packs each core's x shard as xti [128, 16*128*16] fp16
where xti[p, T*2048 + b*16 + g] = x[b, (T*16+g)*128 + p] -- feature-on-
partition, grouped into 16 bank tiles T of 2048 features; within a tile
batch-major (b outer, 16 feature-blocks g inner) so both the DMA (4KB/
partition contiguous) and the DVE broadcast multiply (innermost stride-1
=> 2x fp16 mode) are efficient.

Per bank tile (2048 features -> 512 outputs on 128 partitions):
  - DVE: ONE tensor_tensor multiply x *= w via a step-0 broadcast AP on
    the [128, 16] weight slice (2x_1p fp16 mode).
  - PE: ONE K=4 bias matmul (lhsT = twk4 slice [4, 128], rhs = t4 =
    kron(I4, t) [4, 512]) opens the whole PSUM bank with tw[o]*t[b]
    (start=True, M=128, tile_position (0,0)); then 4 reduce matmuls
    (lhsT = 0/1 block-diagonal ones01 [128, 32], N=512, tile_position
    (0, 32m)) accumulate the segment sums (start=False, stop=True).
    The first 8 bias matmuls are issued up front: they warm the PE
    p-state during the DMA ramp and shrink the steady-state PE load.
  - ACT copies the bank PSUM->SBUF casting to fp16.
  - All 16 output DMAs are issued on the SAME SP HWDGE ring as the x
    loads, AFTER them in program order: ring FIFO ordering defers every
    output transfer past the last x load, so the x stream (the critical
    input) finishes ~6us earlier and outputs drain back-to-back at the
    end (16 out_sb tiles stay live; SBUF is plentiful).
  - Host casts back to fp32 and un-permutes.

Const DMAs are merged into two (one [128, 288] fp16, one [4, 2560]
fp16). A post-pass moves excess semaphore waits onto NoOps (walrus fits
only one wait on several instruction structs).
"""

import sys

if "/opt/trn_rl_repo" not in sys.path:
    sys.path.insert(0, "/opt/trn_rl_repo")

import numpy as np

B, NIN, NOUT, BF = 1024, 32768, 8192, 4
NC = 8
BS = B // NC  # 128 batch rows per core
FBLK = 128  # features per block (partition dim)
NBLK = NIN // FBLK  # 256 feature blocks
TBLK = 16  # feature blocks per bank tile
NT = NBLK // TBLK  # 16 bank tiles
TFREE = BS * TBLK  # 2048 free elems per tile per partition
NGQ = NBLK // 4  # 64 128-output groups
PRE_BIAS = 8  # bias matmuls issued up front (= PSUM banks)

_cache = {}


def _build(reps=1):
    import concourse.bass as bass
    import concourse.mybir as mybir
    from concourse.tile import TileContext

    f16 = mybir.dt.float16
    f32 = mybir.dt.float32
    nc = bass.Bass()
    xti = nc.declare_dram_parameter("xti", [FBLK, NT * TFREE], f16, isOutput=False)
    # cst = [wmat fp16 [128, 256] | ones01 [128, 32]]
    cst = nc.declare_dram_parameter("cst", [FBLK, NBLK + 32], f16, isOutput=False)
    # cst4 = [twk4 [4, NT*128] | t4 [4, 4*BS]]
    # twk4[k, T*128 + m*32 + p'] = tw[((T*4+k)*4 + m)*32 + p']
    # t4 = kron(I4, t): t4[k, gq_l*BS + b] = (k == gq_l) * t[b]
    cst4 = nc.declare_dram_parameter(
        "cst4", [4, NT * FBLK + 4 * BS], f16, isOutput=False
    )
    out_dev = nc.declare_dram_parameter(
        "out_dev", [FBLK, NGQ * BS], f16, isOutput=True
    )

    with TileContext(nc) as tc:
        with (
            tc.tile_pool(name="const", bufs=1) as cpool,
            tc.tile_pool(name="stream", bufs=8) as spool,
            tc.tile_pool(name="osbp", bufs=7) as opool,
            tc.tile_pool(name="osbs", bufs=2) as ospool,
            tc.tile_pool(name="ps", bufs=PRE_BIAS, space="PSUM") as ppool,
        ):
            # ring order [x0, cst, cst4, x1..]: the tiny const transfers
            # hide behind x0's issue latency instead of bubbling the head
            x_tile0 = spool.tile([FBLK, BS, TBLK], f16, tag="x")
            nc.sync.dma_start(
                out=x_tile0[:].rearrange("p b g -> p (b g)"),
                in_=xti[:, 0:TFREE],
            )
            cst_sb = cpool.tile([FBLK, NBLK + 32], f16)
            nc.sync.dma_start(out=cst_sb[:], in_=cst[:])
            cst4_sb = cpool.tile([4, NT * FBLK + 4 * BS], f16)
            nc.sync.dma_start(out=cst4_sb[:], in_=cst4[:])
            wmat_sb = cst_sb[:, :NBLK]
            ones01_sb = cst_sb[:, NBLK : NBLK + 32]
            t4_sb = cst4_sb[:, NT * FBLK :]

            for rep in range(reps):
                if rep > 0:
                    # benchmark mode: serialize reps so the rep-count
                    # differential measures the full single-shot makespan
                    # (ramp + stream + drain), not pipelined steady state
                    tc.strict_bb_all_engine_barrier()
                ps_tiles = []
                out_tiles = []
                for T in range(PRE_BIAS):
                    ps = ppool.tile([FBLK, 4, BS], f32, tag="ps")
                    nc.tensor.matmul(
                        ps[:],
                        cst4_sb[:, T * FBLK : (T + 1) * FBLK],
                        t4_sb,
                        start=True,
                        stop=False,
                        tile_position=(0, 0),
                        skip_group_check=True,
                    )
                    ps_tiles.append(ps)

                for T in range(NT):
                    if T == 0 and rep == 0:
                        x_tile = x_tile0
                    else:
                        x_tile = spool.tile([FBLK, BS, TBLK], f16, tag="x")
                        nc.sync.dma_start(
                            out=x_tile[:].rearrange("p b g -> p (b g)"),
                            in_=xti[:, T * TFREE : (T + 1) * TFREE],
                        )
                    w_b = (
                        wmat_sb[:, T * TBLK : (T + 1) * TBLK]
                        .unsqueeze(1)
                        .broadcast_to([FBLK, BS, TBLK])
                    )
                    nc.vector.tensor_tensor(
                        x_tile[:], x_tile[:], w_b, op=mybir.AluOpType.mult
                    )
                    if T < PRE_BIAS:
                        ps = ps_tiles[T]
                    else:
                        ps = ppool.tile([FBLK, 4, BS], f32, tag="ps")
                        nc.tensor.matmul(
                            ps[:],
                            cst4_sb[:, T * FBLK : (T + 1) * FBLK],
                            t4_sb,
                            start=True,
                            stop=False,
                            tile_position=(0, 0),
                            skip_group_check=True,
                        )
                    for m in range(4):
                        rhs = x_tile[:, :, m::4].rearrange("p b gq -> p gq b")
                        nc.tensor.matmul(
                            ps[32 * m : 32 * (m + 1), :, :],
                            ones01_sb,
                            rhs,
                            start=False,
                            stop=True,
                            tile_position=(0, 32 * m),
                            skip_group_check=True,
                        )
                    # pack output pairs (T0..13) so each out DMA's 728ns
                    # transfer outpaces the 650ns SEQ-issue/HWDGE cadence;
                    # keep T14/T15 single so the tail isn't gated on both
                    if T < NT - 2:
                        if T % 2 == 0:
                            out_sb = opool.tile([FBLK, 2, 4 * BS], f16, tag="osb")
                            out_tiles.append(out_sb)
                        dst = out_tiles[-1][:, T % 2, :]
                    else:
                        out_sb = ospool.tile([FBLK, 4 * BS], f16, tag="osbs")
                        out_tiles.append(out_sb)
                        dst = out_sb[:]
                    nc.scalar.copy(
                        out=dst, in_=ps[:].rearrange("p q n -> p (q n)")
                    )
                for j in range((NT - 2) // 2):
                    nc.sync.dma_start(
                        out=out_dev[:, j * 8 * BS : (j + 1) * 8 * BS],
                        in_=out_tiles[j][:].rearrange("p two n -> p (two n)"),
                    )
                for T in (NT - 2, NT - 1):
                    nc.sync.dma_start(
                        out=out_dev[:, T * 4 * BS : (T + 1) * 4 * BS],
                        in_=out_tiles[7 + T - (NT - 2)][:],
                    )
    return nc


def _legalize_waits(nc):
    """Walrus codegen only fits one sync-wait on several instruction
    structs (matmul load-weights, tensor-scalar, nop/drain ...). Move
    excess waits onto same-engine NoOps inserted right before."""
    import concourse.mybir as mybir

    for fn in nc.m.functions:
        for blk in fn.blocks:
            new_insts = []
            for inst in blk.instructions:
                si = inst.sync_info
                if si is not None and len(si.on_wait) > 1:
                    waits = list(si.on_wait)
                    for k, w in enumerate(waits[:-1]):
                        new_insts.append(
                            mybir.InstNoOp(
                                name=f"{inst.name}-nw{k}",
                                ins=[],
                                outs=[],
                                engine=inst.engine,
                                sync_info=mybir.SyncInfo(
                                    on_wait=[w], on_update=[]
                                ),
                            )
                        )
                    inst.sync_info = mybir.SyncInfo(
                        on_wait=[waits[-1]], on_update=list(si.on_update)
                    )
                new_insts.append(inst)
            blk.instructions = new_insts


def get_nc():
    if "nc" not in _cache:
        nc = _build()
        _legalize_waits(nc)
        _cache["nc"] = nc
    return _cache["nc"]


def make_in_maps(x, t, weight_vals, t_weights):
    x = np.asarray(x, dtype=np.float32)
    t = np.ascontiguousarray(np.asarray(t, dtype=np.float32))
    w = np.asarray(weight_vals, dtype=np.float32)
    tw = np.asarray(t_weights, dtype=np.float32).reshape(NOUT)
    wmat = w.reshape(NBLK, FBLK).T.astype(np.float16)  # [p, g]
    ones01 = np.zeros((FBLK, 32), dtype=np.float16)
    ones01[np.arange(FBLK), np.arange(FBLK) // BF] = 1.0
    cst = np.ascontiguousarray(np.concatenate([wmat, ones01], axis=1))
    # twk4[k, T, m, p'] = tw[((T*4+k)*4 + m)*32 + p']
    twk4 = (
        tw.reshape(NT, 4, 4, 32)  # [T, k, m, p']
        .transpose(1, 0, 2, 3)  # [k, T, m, p']
        .reshape(4, NT * FBLK)
        .astype(np.float16)
    )
    in_maps = []
    for i in range(NC):
        xs = x[i * BS : (i + 1) * BS]  # [128, 32768]
        # xti[p, T*2048 + b*16 + g] = xs[b, (T*16+g)*128 + p]
        xti = np.ascontiguousarray(
            xs.reshape(BS, NT, TBLK, FBLK)
            .transpose(3, 1, 0, 2)
            .reshape(FBLK, NT * TFREE)
            .astype(np.float16)
        )
        t4 = np.kron(
            np.eye(4, dtype=np.float32), t[i * BS : (i + 1) * BS]
        ).astype(np.float16)
        cst4 = np.ascontiguousarray(np.concatenate([twk4, t4], axis=1))
        in_maps.append({"xti": xti, "cst": cst, "cst4": cst4})
    return in_maps


def _unpack_out(out_dev):
    # out_dev [128, 64*128] with dims [pi, (gq, b)]; o = gq*128 + pi
    o = np.asarray(out_dev).astype(np.float32)
    o = o.reshape(FBLK, NGQ, BS).transpose(2, 1, 0)  # [b, gq, pi]
    return np.ascontiguousarray(o.reshape(BS, NOUT))


def _get_runner():
    """Cached jitted shard_map runner (avoids per-call re-tracing that
    run_bass_kernel_spmd's axon redirect pays)."""
    if "runner" in _cache:
        return _cache["runner"]
    import jax
    from jax.experimental.shard_map import shard_map
    from jax.sharding import Mesh, NamedSharding, PartitionSpec

    import concourse.mybir as mybir
    from concourse import bass2jax
    from concourse.bass2jax import _bass_exec_p, partition_id_tensor

    bass2jax.install_neuronx_cc_hook()
    nc = get_nc()
    partition_name = nc.partition_id_tensor.name if nc.partition_id_tensor else None
    in_names, out_names, out_avals, zero_outs = [], [], [], []
    for alloc in nc.m.functions[0].allocations:
        if not isinstance(alloc, mybir.MemoryLocationSet):
            continue
        name = alloc.memorylocations[0].name
        if alloc.kind == "ExternalInput":
            if name != partition_name:
                in_names.append(name)
        elif alloc.kind == "ExternalOutput":
            shape = tuple(alloc.tensor_shape)
            dtype = mybir.dt.np(alloc.dtype)
            out_names.append(name)
            out_avals.append(jax.core.ShapedArray(shape, dtype))
            zero_outs.append(np.zeros(shape, dtype))
    n_params = len(in_names)
    n_outs = len(out_avals)
    all_in_names = list(in_names) + out_names
    if partition_name is not None:
        all_in_names.append(partition_name)

    def _body(*args):
        operands = list(args)
        if partition_name is not None:
            operands.append(partition_id_tensor())
        outs = _bass_exec_p.bind(
            *operands,
            out_avals=tuple(out_avals),
            in_names=tuple(all_in_names),
            out_names=tuple(out_names),
            lowering_input_output_aliases=(),
            sim_require_finite=True,
            sim_require_nnan=True,
            nc=nc,
        )
        return tuple(outs)

    devices = jax.devices()[:NC]
    mesh = Mesh(np.asarray(devices), ("core",))
    in_specs = (PartitionSpec("core"),) * (n_params + n_outs)
    out_specs = (PartitionSpec("core"),) * n_outs
    donate = tuple(range(n_params, n_params + n_outs))
    fn = jax.jit(
        shard_map(
            _body, mesh=mesh, in_specs=in_specs, out_specs=out_specs,
            check_rep=False,
        ),
        donate_argnums=donate,
        keep_unused=True,
    )
    sharding = NamedSharding(mesh, PartitionSpec("core"))
    concat_zeros = [
        np.zeros((NC * z.shape[0], *z.shape[1:]), z.dtype) for z in zero_outs
    ]

    def run(in_maps):
        concat_in = [
            np.concatenate([np.asarray(m[nm]) for m in in_maps], axis=0)
            for nm in in_names
        ]
        in_dev = [jax.device_put(a, sharding) for a in concat_in]
        zs = [jax.device_put(z, sharding) for z in concat_zeros]
        outs = fn(*in_dev, *zs)
        out = np.asarray(outs[0])  # [NC*FBLK, NGQ*BS]
        return out.reshape(NC, FBLK, NGQ * BS)

    _cache["runner"] = run
    return run


def kernel(x, t, weight_vals, t_weights):
    in_maps = make_in_maps(x, t, weight_vals, t_weights)
    try:
        run = _get_runner()
        per_core = run(in_maps)
        return np.ascontiguousarray(
            np.concatenate([_unpack_out(per_core[c]) for c in range(NC)], axis=0)
        )
    except Exception:
        from concourse.bass_utils import run_bass_kernel_spmd

        nc = get_nc()
        res = run_bass_kernel_spmd(nc, in_maps, list(range(NC)))
        return np.ascontiguousarray(
            np.concatenate([_unpack_out(r["out_dev"]) for r in res.results], axis=0)
        )
